# revision 9
# baseline (speedup 1.0000x reference)
"""BiRNN language-model kernel for 8 Trainium2 NeuronCores.

Problem: X = lookup[input_batch]  (S=128, B=32, EMB=32)
         forward + backward Elman scans (HID=8) producing shifted state
         tables Hf_table / Hb_table, concat -> H [S, B, 16],
         logits = H @ weight_o + bias_o  (V=32000), out = log_softmax.

Sharding: data-parallel over batch. Each of the 8 cores owns B_local=4
sequences (512 tokens) and produces its own [512, 32000] f32 shard;
the host reassembles [S, B, V]. No collectives.

Device-side structure (per core):
  * SCAN sbuf tensor [49, 512]: rows 0-7 fwd hidden state (column t =
    state BEFORE consuming token t), rows 8-15 bwd hidden state (same
    token-order convention; the bwd chain walks columns high->low),
    row 16 = ones, rows 17-48 = X^T (gathered embeddings, transposed).
    One PE matmul ([49,8] stationary mat folding W_h, W_x and biases)
    plus one ACT tanh per direction per tick.
  * Rows 0-16 of SCAN are then directly the [17, n_tok] lhsT of the
    output projection (15..0 states + ones row for bias_o).
  * Projection is two-pass per 128-token tile: pass 1 computes logits
    per 500-wide vocab chunk into PSUM and runs exp in-place with
    accum_out to get sum(exp) (logits are bounded ~+-0.1 so the
    max-subtraction of a stable log_softmax is unnecessary); pass 2
    recomputes the chunk and DVE does (logit - ln(sum)) into SBUF
    staging, DMA'd out as 8 MB transfers.
"""

import numpy as np
from contextlib import ExitStack

import concourse.bass as bass
import concourse.bacc as bacc
import concourse.mybir as mybir
import concourse.tile as tile
from concourse.bass_utils import run_bass_kernel_spmd
from concourse.masks import make_identity

F32 = mybir.dt.float32
BF16 = mybir.dt.bfloat16
I32 = mybir.dt.int32
AF = mybir.ActivationFunctionType

S, B, V, EMB, HID = 128, 32, 32000, 32, 8
NCORES = 8
BL = B // NCORES            # 4 sequences per core
T = S * BL                  # 512 tokens per core
NT = T // 128               # 4 token tiles of 128
CH = 500                    # vocab chunk width (<= 500 fits a PSUM bank with slack)
NCH = V // CH               # 64 chunks
GCH = 2                     # chunks per PSUM group (one [128,1024] 2-bank tile)
NGRP = NCH // GCH           # 32 groups
QW = 8000                   # staging quarter width
GRP_PER_Q = NGRP // 4       # 8 groups per staging quarter


def _build_program():
    nc = bacc.Bacc("TRN2", target_bir_lowering=False, debug=False,
                   num_devices=NCORES)

    idx_d = nc.dram_tensor("idx", [128, NT], I32, kind="ExternalInput")
    lookup_d = nc.dram_tensor("lookup", [V, EMB], F32, kind="ExternalInput")
    wf_d = nc.dram_tensor("wf", [128, HID], F32, kind="ExternalInput")
    wb_d = nc.dram_tensor("wb", [128, HID], F32, kind="ExternalInput")
    h0_d = nc.dram_tensor("h0", [HID, 2], F32, kind="ExternalInput")
    perm_d = nc.dram_tensor("perm", [128, 17], F32, kind="ExternalInput")
    wo_d = nc.dram_tensor("wo", [17, V], BF16, kind="ExternalInput")
    out_d = nc.dram_tensor("out", [T, V], F32, kind="ExternalOutput")

    # scan tensor row layout (compute accesses must start at partition
    # 0/32/64/96): rows 0-7 fwd state, rows 32-39 bwd state, row 64 ones,
    # rows 96-127 X^T; everything else stays zero.
    RF, RB, RONE, RX = 0, 32, 64, 96

    with tile.TileContext(nc) as tc, ExitStack() as ctx:
        cpool = ctx.enter_context(tc.tile_pool(name="const", bufs=1))

        scan = cpool.tile([128, T], F32)         # the scan tensor
        ident = cpool.tile([128, 128], F32)
        wf_sb = cpool.tile([128, HID], F32)
        wb_sb = cpool.tile([128, HID], F32)
        perm_sb = cpool.tile([128, 17], F32)
        wo_sb = cpool.tile([17, V], BF16)
        idx_sb = cpool.tile([128, NT], I32)
        h0_sb = cpool.tile([HID, 2], F32)
        lns_sb = cpool.tile([128, NT], F32)      # per-tile ln(sumexp)
        ht16 = cpool.tile([17, T], BF16)         # [Hf; Hb; ones] as bf16

        # ---- load inputs ----
        nc.sync.dma_start(out=wf_sb[:], in_=wf_d[:])
        nc.sync.dma_start(out=wb_sb[:], in_=wb_d[:])
        nc.sync.dma_start(out=wo_sb[:], in_=wo_d[:])
        nc.sync.dma_start(out=idx_sb[:], in_=idx_d[:])
        nc.sync.dma_start(out=h0_sb[:], in_=h0_d[:])
        nc.sync.dma_start(out=perm_sb[:], in_=perm_d[:])
        make_identity(nc, ident[:])

        # ---- init scan tensor ----
        nc.vector.memset(scan[:, :], 0.0)
        nc.vector.memset(scan[RONE:RONE + 1, :], 1.0)
        # fwd initial state at column block 0, bwd initial at the last block
        nc.vector.tensor_copy(out=scan[RF:RF + HID, 0:BL],
                              in_=h0_sb[:, 0:1].to_broadcast([HID, BL]))
        nc.vector.tensor_copy(out=scan[RB:RB + HID, (S - 1) * BL:S * BL],
                              in_=h0_sb[:, 1:2].to_broadcast([HID, BL]))

        # ---- gather embeddings + transpose into scan rows RX:RX+32 ----
        with tc.tile_pool(name="xsetup", bufs=2) as xpool, \
             tc.tile_pool(name="xpsum", bufs=2, space="PSUM") as xppool:
            for t in range(NT):
                xr = xpool.tile([128, EMB], F32, tag="xrows")
                nc.gpsimd.indirect_dma_start(
                    out=xr[:], out_offset=None, in_=lookup_d[:],
                    in_offset=bass.IndirectOffsetOnAxis(
                        ap=idx_sb[:, t:t + 1], axis=0))
                xp = xppool.tile([EMB, 128], F32, tag="xps")
                nc.tensor.transpose(out=xp[:], in_=xr[:], identity=ident[:])
                nc.vector.tensor_copy(
                    out=scan[RX:RX + EMB, t * 128:(t + 1) * 128], in_=xp[:])

        # ---- the two sequential scans (127 ticks each, interleaved) ----
        with tc.tile_pool(name="scanpsum", bufs=2, space="PSUM") as spsum:
            for t in range(S - 1):
                j = S - 1 - t          # bwd token
                pf = spsum.tile([HID, BL], F32, tag="pf")
                nc.tensor.matmul(out=pf[:], lhsT=wf_sb[:],
                                 rhs=scan[:, t * BL:(t + 1) * BL],
                                 start=True, stop=True)
                nc.scalar.activation(
                    out=scan[RF:RF + HID, (t + 1) * BL:(t + 2) * BL],
                    in_=pf[:], func=AF.Tanh)
                pb = spsum.tile([HID, BL], F32, tag="pb")
                nc.tensor.matmul(out=pb[:], lhsT=wb_sb[:],
                                 rhs=scan[:, j * BL:(j + 1) * BL],
                                 start=True, stop=True)
                nc.scalar.activation(
                    out=scan[RB:RB + HID, (j - 1) * BL:j * BL],
                    in_=pb[:], func=AF.Tanh)

        # ---- assemble [Hf; Hb; ones] via permutation matmul, cast bf16 ----
        with tc.tile_pool(name="htpsum", bufs=1, space="PSUM") as htp:
            htps = htp.tile([17, T], F32)
            nc.tensor.matmul(out=htps[:], lhsT=perm_sb[:], rhs=scan[:, :],
                             start=True, stop=True)
            nc.vector.tensor_copy(out=ht16[:], in_=htps[:])

        # ---- output projection + log_softmax ----
        with tc.tile_pool(name="p1psum", bufs=2, space="PSUM") as p1p, \
             tc.tile_pool(name="p2psum", bufs=2, space="PSUM") as p2p, \
             tc.tile_pool(name="stg", bufs=2) as stgp, \
             tc.tile_pool(name="small", bufs=2) as smallp:

            def wo_slice(j):
                return wo_sb[:, CH * j:CH * (j + 1)]

            for tl in range(NT):
                lhsT = ht16[0:17, tl * 128:(tl + 1) * 128]
                partials = smallp.tile([128, NGRP], F32, tag="partials")
                # pass 1: exp-accumulate all vocab chunks
                for g in range(NGRP):
                    grp = p1p.tile([128, 1024], F32, tag="g1")
                    for c in range(GCH):
                        nc.tensor.matmul(out=grp[:, 512 * c:512 * c + CH],
                                         lhsT=lhsT, rhs=wo_slice(g * GCH + c),
                                         start=True, stop=True)
                    ap3 = grp[:].rearrange("p (c x) -> p c x", c=GCH)[:, :, 0:CH]
                    nc.scalar.activation(out=ap3, in_=ap3, func=AF.Exp,
                                         accum_out=partials[:, g:g + 1])
                sume = smallp.tile([128, 1], F32, tag="sume")
                nc.vector.tensor_reduce(out=sume[:], in_=partials[:],
                                        axis=mybir.AxisListType.X,
                                        op=mybir.AluOpType.add)
                nc.scalar.activation(out=lns_sb[:, tl:tl + 1], in_=sume[:],
                                     func=AF.Ln)
                # pass 2: recompute logits, subtract ln(sum), stage + DMA out
                for q in range(4):
                    stg = stgp.tile([128, QW], F32, tag="stg")
                    for gg in range(GRP_PER_Q):
                        g = q * GRP_PER_Q + gg
                        grp = p2p.tile([128, 1024], F32, tag="g2")
                        for c in range(GCH):
                            nc.tensor.matmul(out=grp[:, 512 * c:512 * c + CH],
                                             lhsT=lhsT,
                                             rhs=wo_slice(g * GCH + c),
                                             start=True, stop=True)
                        src3 = grp[:].rearrange("p (c x) -> p c x",
                                                c=GCH)[:, :, 0:CH]
                        dst3 = stg[:, gg * 1000:(gg + 1) * 1000].rearrange(
                            "p (c x) -> p c x", c=GCH)
                        nc.vector.tensor_scalar(
                            out=dst3, in0=src3,
                            scalar1=lns_sb[:, tl:tl + 1], scalar2=None,
                            op0=mybir.AluOpType.subtract)
                    nc.sync.dma_start(
                        out=out_d[tl * 128:(tl + 1) * 128,
                                  q * QW:(q + 1) * QW],
                        in_=stg[:])

    nc.compile()
    return nc


_NC = None


def _get_program():
    global _NC
    if _NC is None:
        _NC = _build_program()
    return _NC


def _make_in_maps(inputs):
    input_batch = np.asarray(inputs["input_batch"])
    lookup = np.asarray(inputs["lookup"], dtype=np.float32)
    weight_xf = np.asarray(inputs["weight_xf"], dtype=np.float32)
    weight_hf = np.asarray(inputs["weight_hf"], dtype=np.float32)
    weight_xb = np.asarray(inputs["weight_xb"], dtype=np.float32)
    weight_hb = np.asarray(inputs["weight_hb"], dtype=np.float32)
    weight_o = np.asarray(inputs["weight_o"], dtype=np.float32)
    Hf = np.asarray(inputs["Hf"], dtype=np.float32)
    Hb = np.asarray(inputs["Hb"], dtype=np.float32)
    bias_x = np.asarray(inputs["bias_x"], dtype=np.float32)
    bias_hf = np.asarray(inputs["bias_hf"], dtype=np.float32)
    bias_hb = np.asarray(inputs["bias_hb"], dtype=np.float32)
    bias_o = np.asarray(inputs["bias_o"], dtype=np.float32)

    RF, RB, RONE, RX = 0, 32, 64, 96
    wf = np.zeros((128, HID), np.float32)
    wf[RF:RF + HID] = weight_hf
    wf[RONE] = bias_x + bias_hf
    wf[RX:RX + EMB] = weight_xf
    wb = np.zeros((128, HID), np.float32)
    wb[RB:RB + HID] = weight_hb
    wb[RONE] = bias_x + bias_hb
    wb[RX:RX + EMB] = weight_xb
    h0 = np.stack([Hf, Hb], axis=1).astype(np.float32)      # [8, 2]

    perm = np.zeros((128, 17), np.float32)
    for m in range(HID):
        perm[RF + m, m] = 1.0
        perm[RB + m, HID + m] = 1.0
    perm[RONE, 16] = 1.0

    import ml_dtypes
    wo = np.concatenate([weight_o, bias_o[None, :]], axis=0)  # [17, V]
    wo = wo.astype(ml_dtypes.bfloat16)

    in_maps = []
    for c in range(NCORES):
        flat = np.ascontiguousarray(
            input_batch[:, c * BL:(c + 1) * BL]).reshape(-1)  # token r = s*BL+b
        idx = np.ascontiguousarray(
            flat.reshape(NT, 128).T).astype(np.int32)         # [128, NT]
        in_maps.append({
            "idx": idx, "lookup": lookup, "wf": wf, "wb": wb,
            "h0": h0, "wo": wo, "perm": perm,
        })
    return in_maps


def _assemble(results):
    out = np.empty((S, B, V), np.float32)
    for c in range(NCORES):
        out[:, c * BL:(c + 1) * BL, :] = results[c]["out"].reshape(S, BL, V)
    return out


def run(inputs, **kwargs):
    """Run on hardware; returns (full_output, BassKernelResults)."""
    nc = _get_program()
    in_maps = _make_in_maps(inputs)
    res = run_bass_kernel_spmd(nc, in_maps, core_ids=list(range(NCORES)),
                               **kwargs)
    return _assemble(res.results), res


def kernel(**inputs) -> np.ndarray:
    out, _ = run(inputs)
    return out


# revision 16
# speedup vs baseline: 1.1621x; 1.1621x over previous
"""BiRNN language-model kernel for 8 Trainium2 NeuronCores.

Problem: X = lookup[input_batch]  (S=128, B=32, EMB=32)
         forward + backward Elman scans (HID=8) producing shifted state
         tables Hf_table / Hb_table, concat -> H [S, B, 16],
         logits = H @ weight_o + bias_o  (V=32000), out = log_softmax.

Sharding: data-parallel over batch. Each of the 8 cores owns B_local=4
sequences (512 tokens) and produces its own [512, 32000] f32 shard;
the host reassembles [S, B, V]. No collectives.

Device-side structure (per core):
  * SCAN sbuf tensor [49, 512]: rows 0-7 fwd hidden state (column t =
    state BEFORE consuming token t), rows 8-15 bwd hidden state (same
    token-order convention; the bwd chain walks columns high->low),
    row 16 = ones, rows 17-48 = X^T (gathered embeddings, transposed).
    One PE matmul ([49,8] stationary mat folding W_h, W_x and biases)
    plus one ACT tanh per direction per tick.
  * Rows 0-16 of SCAN are then directly the [17, n_tok] lhsT of the
    output projection (15..0 states + ones row for bias_o).
  * Projection is two-pass per 128-token tile: pass 1 computes logits
    per 500-wide vocab chunk into PSUM and runs exp in-place with
    accum_out to get sum(exp) (logits are bounded ~+-0.1 so the
    max-subtraction of a stable log_softmax is unnecessary); pass 2
    recomputes the chunk and DVE does (logit - ln(sum)) into SBUF
    staging, DMA'd out as 8 MB transfers.
"""

import numpy as np
from contextlib import ExitStack

import concourse.bass as bass
import concourse.bacc as bacc
import concourse.mybir as mybir
import concourse.tile as tile
from concourse.bass_utils import run_bass_kernel_spmd
from concourse.masks import make_identity



F32 = mybir.dt.float32
BF16 = mybir.dt.bfloat16
I32 = mybir.dt.int32
AF = mybir.ActivationFunctionType

S, B, V, EMB, HID = 128, 32, 32000, 32, 8
NCORES = 8
BL = B // NCORES            # 4 sequences per core
T = S * BL                  # 512 tokens per core
NT = T // 128               # 4 token tiles of 128
CH = 500                    # vocab chunk width (<= 500 fits a PSUM bank with slack)
NCH = V // CH               # 64 chunks
GCH = 2                     # chunks per PSUM group (one [128,1024] 2-bank tile)
NGRP = NCH // GCH           # 32 groups
QW = 8000                   # staging quarter width
GRP_PER_Q = NGRP // 4       # 8 groups per staging quarter


def _build_program():
    nc = bacc.Bacc("TRN2", target_bir_lowering=False, debug=False,
                   num_devices=NCORES)

    idx_d = nc.dram_tensor("idx", [128, NT], I32, kind="ExternalInput")
    lookup_d = nc.dram_tensor("lookup", [V, EMB], F32, kind="ExternalInput")
    wf_d = nc.dram_tensor("wf", [128, HID], F32, kind="ExternalInput")
    wb_d = nc.dram_tensor("wb", [128, HID], F32, kind="ExternalInput")
    h0_d = nc.dram_tensor("h0", [HID, 2], F32, kind="ExternalInput")
    perm_d = nc.dram_tensor("perm", [128, 17], F32, kind="ExternalInput")
    # wo is zero-padded to K=128 rows: only rows 0-16 are data. The pad
    # makes every projection matmul drive all 128 PE rows, which keeps the
    # PE activity monitor in the 2.4 GHz state (K=17 matmuls measured stuck
    # at the cold 1.2 GHz clock).
    wo_d = nc.dram_tensor("wo", [128, V], BF16, kind="ExternalInput")
    out_d = nc.dram_tensor("out", [T, V], F32, kind="ExternalOutput")

    # scan tensor row layout (compute accesses must start at partition
    # 0/32/64/96): rows 0-7 fwd state, rows 32-39 bwd state, row 64 ones,
    # rows 96-127 X^T; everything else stays zero.
    RF, RB, RONE, RX = 0, 32, 64, 96

    with tile.TileContext(nc) as tc, ExitStack() as ctx:
        cpool = ctx.enter_context(tc.tile_pool(name="const", bufs=1))

        scan = cpool.tile([128, T], F32)         # the scan tensor
        ident = cpool.tile([128, 128], F32)
        wf_sb = cpool.tile([128, HID], F32)
        wb_sb = cpool.tile([128, HID], F32)
        perm_sb = cpool.tile([128, 17], F32)
        wo_sb = cpool.tile([128, V], BF16)
        idx_sb = cpool.tile([128, NT], I32)
        h0_sb = cpool.tile([HID, 2], F32)
        lns_sb = cpool.tile([128, NT], F32)      # per-tile ln(sumexp)
        ht16 = cpool.tile([128, T], BF16)        # [Hf; Hb; ones; 0-pad] bf16

        # ---- load inputs (idx first: the gather chain is on the critical
        # path; wo is issued after the scan is emitted, so its ~8 MB drain
        # overlaps the sequential scan) ----
        nc.sync.dma_start(out=idx_sb[:], in_=idx_d[:])
        nc.sync.dma_start(out=wf_sb[:], in_=wf_d[:])
        nc.sync.dma_start(out=wb_sb[:], in_=wb_d[:])
        nc.sync.dma_start(out=h0_sb[:], in_=h0_d[:])
        nc.sync.dma_start(out=perm_sb[:], in_=perm_d[:])
        make_identity(nc, ident[:])

        # ---- init scan tensor ----
        nc.vector.memset(scan[:, :], 0.0)
        nc.vector.memset(scan[RONE:RONE + 1, :], 1.0)
        # fwd initial state at column block 0, bwd initial at the last block
        nc.vector.tensor_copy(out=scan[RF:RF + HID, 0:BL],
                              in_=h0_sb[:, 0:1].to_broadcast([HID, BL]))
        nc.vector.tensor_copy(out=scan[RB:RB + HID, (S - 1) * BL:S * BL],
                              in_=h0_sb[:, 1:2].to_broadcast([HID, BL]))

        # ---- gather embeddings + transpose into scan rows RX:RX+32 ----
        with tc.tile_pool(name="xsetup", bufs=2) as xpool, \
             tc.tile_pool(name="xpsum", bufs=2, space="PSUM") as xppool:
            for t in range(NT):
                xr = xpool.tile([128, EMB], F32, tag="xrows")
                nc.gpsimd.indirect_dma_start(
                    out=xr[:], out_offset=None, in_=lookup_d[:],
                    in_offset=bass.IndirectOffsetOnAxis(
                        ap=idx_sb[:, t:t + 1], axis=0))
                xp = xppool.tile([EMB, 128], F32, tag="xps")
                nc.tensor.transpose(out=xp[:], in_=xr[:], identity=ident[:])
                nc.vector.tensor_copy(
                    out=scan[RX:RX + EMB, t * 128:(t + 1) * 128], in_=xp[:])

        # ---- the two sequential scans (127 ticks each, interleaved) ----
        with tc.tile_pool(name="scanpsum", bufs=2, space="PSUM") as spsum:
            for t in range(S - 1):
                j = S - 1 - t          # bwd token
                pf = spsum.tile([HID, BL], F32, tag="pf")
                nc.tensor.matmul(out=pf[:], lhsT=wf_sb[:],
                                 rhs=scan[:, t * BL:(t + 1) * BL],
                                 start=True, stop=True)
                nc.scalar.activation(
                    out=scan[RF:RF + HID, (t + 1) * BL:(t + 2) * BL],
                    in_=pf[:], func=AF.Tanh)
                pb = spsum.tile([HID, BL], F32, tag="pb")
                nc.tensor.matmul(out=pb[:], lhsT=wb_sb[:],
                                 rhs=scan[:, j * BL:(j + 1) * BL],
                                 start=True, stop=True)
                nc.scalar.activation(
                    out=scan[RB:RB + HID, (j - 1) * BL:j * BL],
                    in_=pb[:], func=AF.Tanh)

        # wo load: emitted here so its DMA drains during the scan above
        nc.sync.dma_start(out=wo_sb[:], in_=wo_d[:])

        # ---- assemble [Hf; Hb; ones] via permutation matmul, cast bf16 ----
        nc.vector.memset(ht16[:, :], 0.0)
        with tc.tile_pool(name="htpsum", bufs=1, space="PSUM") as htp:
            htps = htp.tile([17, T], F32)
            nc.tensor.matmul(out=htps[:], lhsT=perm_sb[:], rhs=scan[:, :],
                             start=True, stop=True)
            nc.vector.tensor_copy(out=ht16[0:17, :], in_=htps[:])

        # ---- output projection + log_softmax ----
        with tc.tile_pool(name="p1psum", bufs=2, space="PSUM") as p1p, \
             tc.tile_pool(name="p2psum", bufs=2, space="PSUM") as p2p, \
             tc.tile_pool(name="stg", bufs=2) as stgp, \
             tc.tile_pool(name="small", bufs=2) as smallp:

            def wo_slice(j):
                return wo_sb[:, CH * j:CH * (j + 1)]

            for tl in range(NT):
                lhsT = ht16[:, tl * 128:(tl + 1) * 128]
                partials = smallp.tile([128, NGRP], F32, tag="partials")
                # pass 1: exp-accumulate all vocab chunks
                for g in range(NGRP):
                    grp = p1p.tile([128, 1024], F32, tag="g1")
                    for c in range(GCH):
                        nc.tensor.matmul(out=grp[:, 512 * c:512 * c + CH],
                                         lhsT=lhsT, rhs=wo_slice(g * GCH + c),
                                         start=True, stop=True)
                    ap3 = grp[:].rearrange("p (c x) -> p c x", c=GCH)[:, :, 0:CH]
                    nc.scalar.activation(out=ap3, in_=ap3, func=AF.Exp,
                                         accum_out=partials[:, g:g + 1])
                sume = smallp.tile([128, 1], F32, tag="sume")
                nc.vector.tensor_reduce(out=sume[:], in_=partials[:],
                                        axis=mybir.AxisListType.X,
                                        op=mybir.AluOpType.add)
                nc.scalar.activation(out=lns_sb[:, tl:tl + 1], in_=sume[:],
                                     func=AF.Ln)
                # pass 2: recompute logits, subtract ln(sum), stage + DMA out
                for q in range(4):
                    stg = stgp.tile([128, QW], F32, tag="stg")
                    for gg in range(GRP_PER_Q):
                        g = q * GRP_PER_Q + gg
                        grp = p2p.tile([128, 1024], F32, tag="g2")
                        for c in range(GCH):
                            nc.tensor.matmul(out=grp[:, 512 * c:512 * c + CH],
                                             lhsT=lhsT,
                                             rhs=wo_slice(g * GCH + c),
                                             start=True, stop=True)
                        src3 = grp[:].rearrange("p (c x) -> p c x",
                                                c=GCH)[:, :, 0:CH]
                        dst3 = stg[:, gg * 1000:(gg + 1) * 1000].rearrange(
                            "p (c x) -> p c x", c=GCH)
                        nc.vector.tensor_scalar(
                            out=dst3, in0=src3,
                            scalar1=lns_sb[:, tl:tl + 1], scalar2=None,
                            op0=mybir.AluOpType.subtract)
                    nc.sync.dma_start(
                        out=out_d[tl * 128:(tl + 1) * 128,
                                  q * QW:(q + 1) * QW],
                        in_=stg[:])

    nc.compile()
    return nc


_NC = None


def _get_program():
    global _NC
    if _NC is None:
        _NC = _build_program()
    return _NC


def _make_in_maps(inputs):
    input_batch = np.asarray(inputs["input_batch"])
    lookup = np.asarray(inputs["lookup"], dtype=np.float32)
    weight_xf = np.asarray(inputs["weight_xf"], dtype=np.float32)
    weight_hf = np.asarray(inputs["weight_hf"], dtype=np.float32)
    weight_xb = np.asarray(inputs["weight_xb"], dtype=np.float32)
    weight_hb = np.asarray(inputs["weight_hb"], dtype=np.float32)
    weight_o = np.asarray(inputs["weight_o"], dtype=np.float32)
    Hf = np.asarray(inputs["Hf"], dtype=np.float32)
    Hb = np.asarray(inputs["Hb"], dtype=np.float32)
    bias_x = np.asarray(inputs["bias_x"], dtype=np.float32)
    bias_hf = np.asarray(inputs["bias_hf"], dtype=np.float32)
    bias_hb = np.asarray(inputs["bias_hb"], dtype=np.float32)
    bias_o = np.asarray(inputs["bias_o"], dtype=np.float32)

    RF, RB, RONE, RX = 0, 32, 64, 96
    wf = np.zeros((128, HID), np.float32)
    wf[RF:RF + HID] = weight_hf
    wf[RONE] = bias_x + bias_hf
    wf[RX:RX + EMB] = weight_xf
    wb = np.zeros((128, HID), np.float32)
    wb[RB:RB + HID] = weight_hb
    wb[RONE] = bias_x + bias_hb
    wb[RX:RX + EMB] = weight_xb
    h0 = np.stack([Hf, Hb], axis=1).astype(np.float32)      # [8, 2]

    perm = np.zeros((128, 17), np.float32)
    for m in range(HID):
        perm[RF + m, m] = 1.0
        perm[RB + m, HID + m] = 1.0
    perm[RONE, 16] = 1.0

    import ml_dtypes
    wo = np.zeros((128, V), ml_dtypes.bfloat16)
    wo[0:16] = weight_o.astype(ml_dtypes.bfloat16)
    wo[16] = bias_o.astype(ml_dtypes.bfloat16)

    in_maps = []
    for c in range(NCORES):
        flat = np.ascontiguousarray(
            input_batch[:, c * BL:(c + 1) * BL]).reshape(-1)  # token r = s*BL+b
        idx = np.ascontiguousarray(
            flat.reshape(NT, 128).T).astype(np.int32)         # [128, NT]
        in_maps.append({
            "idx": idx, "lookup": lookup, "wf": wf, "wb": wb,
            "h0": h0, "wo": wo, "perm": perm,
        })
    return in_maps


def _assemble(results):
    out = np.empty((S, B, V), np.float32)
    for c in range(NCORES):
        out[:, c * BL:(c + 1) * BL, :] = results[c]["out"].reshape(S, BL, V)
    return out


def run(inputs, **kwargs):
    """Run on hardware; returns (full_output, BassKernelResults)."""
    nc = _get_program()
    in_maps = _make_in_maps(inputs)
    res = run_bass_kernel_spmd(nc, in_maps, core_ids=list(range(NCORES)),
                               **kwargs)
    return _assemble(res.results), res


def kernel(**inputs) -> np.ndarray:
    out, _ = run(inputs)
    return out
